# revision 18
# baseline (speedup 1.0000x reference)
"""Trainium2 Bass kernel for 3D neighborhood attention (sparse_attention).

Problem: q,k [1,40,40,40,48] fp32, rpb [8,3,3,3]; out [1,24,40,40,40].
Per voxel x: logits[h,kk] = scale * <q[x,h,:], k[x+off_kk,h,:]> + rpb[h,kk]
(zero-padded k at boundaries, kk over 3x3x3 offsets), p = softmax over kk,
out[x,h,:] = sum_kk p[h,kk] * off_kk  (constant integer offsets as values).

Sharding: spatial-parallel over H (40 -> 8 slabs of 5), no collectives;
halo rows of k are prepared host-side.

On-core layout (the key idea): SBUF partitions = (w-column j, t-block tb)
with T split into 3 blocks of 14 (+1 halo each side inside the 16-wide
stored block), so ALL 27 neighborhood shifts are pure access-pattern
offsets: dj shifts partitions by 3*dj, di/dl shift the free offset. No
im2col, no on-chip data movement. q/k are fp16 so every DVE op runs in
2x_1P mode. Logits: 27 shifted multiplies (d-outer layout) + a binary
tree over d. rpb is a broadcast add folded before exp (exp on ScalarE,
overlapped per chunk). The PV contraction uses that values are the
constant offsets: out_o = (sum_{+1 slab} E - sum_{-1 slab} E) / sum E,
computed as fp16 slab trees reusing A[a,b] = sum_dl E partial sums.
"""

import numpy as np

import concourse.bass as bass
import concourse.tile as tile
from concourse import bacc, mybir
from concourse.bass_utils import run_bass_kernel_spmd

NH = 8
HD = 6
KS = 3
H = W = T = 40
SCALE = HD**-0.5
N_CORES = 8
SLAB = H // N_CORES          # 5 rows of H per core

TB = 3                       # t-blocks per line
TIN = 14                     # tokens per t-block (3*14 = 42 >= 40)
TQ = 16                      # stored t per block (with halo)
QP = W * TB                  # 120 partitions for q/out
KP = (W + 2) * TB            # 126 partitions for k
QF = HD * SLAB * TIN * NH    # 3360 q free size (d, i, t, h)
KF = HD * (SLAB + 2) * TQ * NH  # 5376 k free size (d, ip, tq, h)
X = SLAB * TIN * NH          # 560 = (i, t, h)
NT = KS**3                   # 27
LF = NT * X                  # 15120 logits free size (kk, i, t, h)

_prog_cache = {}


def _build_program():
    f16 = mybir.dt.float16
    f32 = mybir.dt.float32
    nc = bacc.Bacc("TRN2", target_bir_lowering=False, debug=False,
                   num_devices=N_CORES)
    qs = nc.dram_tensor("qs", [QP, QF], f16, kind="ExternalInput").ap()
    ks = nc.dram_tensor("ks", [KP, KF], f16, kind="ExternalInput").ap()
    ws = nc.dram_tensor("ws", [QP, NT * TIN * NH], f16,
                        kind="ExternalInput").ap()
    out = nc.dram_tensor("out", [QP, 3 * X], f32, kind="ExternalOutput").ap()

    ADD = mybir.AluOpType.add
    EXP = mybir.ActivationFunctionType.Exp

    with tile.TileContext(nc) as tc:
        with (
            tc.tile_pool(name="io", bufs=1) as iop,
            tc.tile_pool(name="work", bufs=1) as wp,
        ):
            qt = iop.tile([QP, QF], f16)
            # one k copy per dj shift (compute-engine APs must start at an
            # aligned partition, so partition-offset views are not allowed)
            kts = [iop.tile([QP, KF], f16, name=f"kt{b}") for b in range(3)]
            wt = iop.tile([QP, NT * TIN * NH], f16)
            nc.sync.dma_start(qt[:], qs[:])
            nc.sync.dma_start(kts[0][:], ks[0:QP])
            nc.sync.dma_start(kts[2][:], ks[6:6 + QP])
            nc.sync.dma_start(kts[1][:], ks[3:3 + QP])
            nc.sync.dma_start(wt[:], ws[:])

            pt = wp.tile([QP, HD * 3 * X], f16)   # products (d, a, i, t, h)
            a3 = wp.tile([QP, 3 * 3 * X], f16)    # tree stage 1 (3 planes)
            b2 = wp.tile([QP, 3 * X], f16)
            cs = wp.tile([QP, 3 * X], f16)        # logits before rpb
            # separate scratch set for the GpSimd-offloaded chunks
            pt2 = wp.tile([QP, HD * 3 * X], f16)
            a32 = wp.tile([QP, 3 * 3 * X], f16)
            b22 = wp.tile([QP, 3 * X], f16)
            cs2 = wp.tile([QP, 3 * X], f16)
            lt = wp.tile([QP, LF], f16)           # logits, exp'd in place

            TH = TIN * NH                          # 112
            KTH = TQ * NH                          # 128
            qm = qt[:].rearrange("p (d i th) -> p d i th", d=HD, i=SLAB)
            kms = [kts[b][:].rearrange("p (d i th) -> p d i th",
                                       d=HD, i=SLAB + 2)
                   for b in range(3)]
            pm = pt[:].rearrange("p (d a i th) -> p d a i th",
                                 d=HD, a=3, i=SLAB)
            pd6 = pt[:].rearrange("p (d x) -> p d x", d=HD)
            a3v = a3[:].rearrange("p (d x) -> p d x", d=3)
            wv = wt[:].rearrange("p (a r th) -> p a r th", a=3, r=9)
            la4 = lt[:].rearrange("p (a r i th) -> p a r i th",
                                  a=3, r=9, i=SLAB)
            cs4 = cs[:].rearrange("p (a i th) -> p a i th",
                                  a=3, i=SLAB)
            lv4 = lt[:].rearrange("p (a r x) -> p a r x", a=3, r=9)
            cs3 = cs[:].rearrange("p (a x) -> p a x", a=3)

            pm2 = pt2[:].rearrange("p (d a i th) -> p d a i th",
                                   d=HD, a=3, i=SLAB)
            pd62 = pt2[:].rearrange("p (d x) -> p d x", d=HD)
            a3v2 = a32[:].rearrange("p (d x) -> p d x", d=3)
            cs42 = cs2[:].rearrange("p (a i th) -> p a i th",
                                    a=3, i=SLAB)

            # chunks (b, dl): 7 on DVE, 2 (b=2, dl=1,2) on GpSimd (runs
            # concurrently; ~3.4x slower per element, so 2/9 of the work)
            for b in range(3):
                for dl in range(3):
                    on_pool = (b == 2 and dl >= 1)
                    eng = nc.gpsimd if on_pool else nc.vector
                    _pm = pm2 if on_pool else pm
                    _pd = pd62 if on_pool else pd6
                    _a3 = a3v2 if on_pool else a3v
                    _b2 = b22 if on_pool else b2
                    _cs = cs2 if on_pool else cs
                    _cs4 = cs42 if on_pool else cs4
                    # products for the 3 kk = (a, b, dl), a in 0..2
                    for a in range(3):
                        kslice = kms[b][:, :, a:a + SLAB,
                                        dl * NH:dl * NH + TH]
                        eng.tensor_mul(_pm[:, :, a], qm, kslice)
                    # tree-reduce over d (outermost): 3+3 -> 3 -> 1
                    eng.tensor_add(_a3[:], _pd[:, 0:3], _pd[:, 3:6])
                    eng.tensor_add(_b2[:], _a3[:, 0], _a3[:, 1])
                    eng.tensor_add(_cs[:], _b2[:], _a3[:, 2])
                    # + rpb (pre-broadcast host-side) -> logits slice
                    r = 3 * b + dl
                    wb = (wv[:, :, r].unsqueeze(2)
                          .broadcast_to([QP, 3, SLAB, TH]))
                    eng.tensor_add(la4[:, :, r], _cs4[:], wb)
                    # exp on ScalarE (overlaps next chunk's DVE work)
                    nc.scalar.activation(lv4[:, :, r], lv4[:, :, r], EXP)

            # PV phase: values are the constant offsets in {-1,0,1}^3.
            # A[a,b] = sum_dl E; S = sum_ab A; N_a/N_b from A slabs;
            # N_l needs its own dl-slab sums.
            evk = lt[:].rearrange("p (a b dl x) -> p a b dl x", a=3, b=3, dl=3)
            # PV scratch lives in the (now idle) product tiles
            a1f = pt[:, 0:9 * X]
            aabf = pt[:, 9 * X:18 * X]
            a1v = a1f.rearrange("p (a b x) -> p a b x", a=3, b=3)
            aabv = aabf.rearrange("p (a b x) -> p a b x", a=3, b=3)
            nc.vector.tensor_add(a1v[:], evk[:, :, :, 0], evk[:, :, :, 1])
            nc.vector.tensor_add(aabv[:], a1v[:], evk[:, :, :, 2])

            aab9 = aabf.rearrange("p (n x) -> p n x", n=9)
            s4 = wp.tile([QP, 4 * X], f16)
            s2 = wp.tile([QP, 2 * X], f16)
            s1 = wp.tile([QP, X], f16)
            st = wp.tile([QP, X], f32)
            s4v = s4[:].rearrange("p (n x) -> p n x", n=4)
            s2v = s2[:].rearrange("p (n x) -> p n x", n=2)
            nc.vector.tensor_add(s4v[:], aab9[:, 0:4], aab9[:, 4:8])
            nc.vector.tensor_add(s2v[:], s4v[:, 0:2], s4v[:, 2:4])
            nc.vector.tensor_add(s1[:], s2v[:, 0], s2v[:, 1])
            nc.vector.tensor_add(st[:], s1[:], aab9[:, 8])

            nbuf = wp.tile([QP, 3 * X], f16)
            nv = nbuf[:].rearrange("p (o x) -> p o x", o=3)

            # N_a = sum_b (A[2,b] - A[0,b])
            da = wp.tile([QP, 3 * X], f16)
            dav = da[:].rearrange("p (n x) -> p n x", n=3)
            na2 = wp.tile([QP, X], f16)
            nc.vector.tensor_sub(dav[:], aabv[:, 2], aabv[:, 0])
            nc.vector.tensor_add(na2[:], dav[:, 0], dav[:, 1])
            nc.vector.tensor_add(nv[:, 0], na2[:], dav[:, 2])

            # N_b = sum_a (A[a,2] - A[a,0])
            db = wp.tile([QP, 3 * X], f16)
            dbv = db[:].rearrange("p (n x) -> p n x", n=3)
            nb2 = wp.tile([QP, X], f16)
            nc.vector.tensor_sub(dbv[:], aabv[:, :, 2], aabv[:, :, 0])
            nc.vector.tensor_add(nb2[:], dbv[:, 0], dbv[:, 1])
            nc.vector.tensor_add(nv[:, 1], nb2[:], dbv[:, 2])

            # N_l = sum_ab (E[.,.,2] - E[.,.,0])
            dlbf = pt2[:, 0:9 * X]
            dl9 = dlbf.rearrange("p (n x) -> p n x", n=9)
            dl4 = wp.tile([QP, 4 * X], f16)
            dl2 = wp.tile([QP, 2 * X], f16)
            dl1 = wp.tile([QP, X], f16)
            dl4v = dl4[:].rearrange("p (n x) -> p n x", n=4)
            dl2v = dl2[:].rearrange("p (n x) -> p n x", n=2)
            dlabv = dlbf.rearrange("p (a b x) -> p a b x", a=3, b=3)
            nc.vector.tensor_sub(dlabv[:], evk[:, :, :, 2], evk[:, :, :, 0])
            nc.vector.tensor_add(dl4v[:], dl9[:, 0:4], dl9[:, 4:8])
            nc.vector.tensor_add(dl2v[:], dl4v[:, 0:2], dl4v[:, 2:4])
            nc.vector.tensor_add(dl1[:], dl2v[:, 0], dl2v[:, 1])
            nc.vector.tensor_add(nv[:, 2], dl1[:], dl9[:, 8])

            sinv = wp.tile([QP, X], f32)
            nc.vector.reciprocal_approx_fast(sinv[:], st[:])

            ot = wp.tile([QP, 3 * X], f32)
            ov = ot[:].rearrange("p (o x) -> p o x", o=3)
            sb = sinv[:].unsqueeze(1).broadcast_to([QP, 3, X])
            nc.vector.tensor_mul(ov[:], nv[:], sb)
            nc.sync.dma_start(out[:], ot[:])

    nc.compile()
    return nc


def _host_prep(q, k, rpb):
    q0 = (np.asarray(q, np.float32)[0] * SCALE)          # [40,40,40,48]
    k0 = np.asarray(k, np.float32)[0]
    rpb = np.asarray(rpb, np.float32)

    # padded k: [H+2, W+2, T+4, 48] (t gets 1 left + 3 right zeros so the
    # tb=2 stored block [27..43) is in range)
    kp = np.zeros((H + 2, W + 2, T + 4, NH * HD), np.float16)
    kp[1:H + 1, 1:W + 1, 1:T + 1] = k0
    # q padded in t to 42
    qp = np.zeros((H, W, TB * TIN, NH * HD), np.float16)
    qp[:, :, :T] = q0

    # rpb -> w[kk=(a,b,dl), t, h], pre-broadcast over t, replicated
    w_np = rpb.transpose(1, 2, 3, 0).astype(np.float16)      # [3,3,3,8]
    w_np = np.broadcast_to(w_np[:, :, :, None, :],
                           (3, 3, 3, TIN, NH)).reshape(NT * TIN * NH)
    w_rep = np.broadcast_to(w_np, (QP, NT * TIN * NH)).copy()

    in_maps = []
    for c in range(N_CORES):
        i0 = c * SLAB
        # q_sb[j*3+tb, (d, a=1, i, t, h)] = qp[i0+i, j, tb*14+t, h*6+d]
        q_sb = np.zeros((QP, QF), np.float16)
        for tb in range(TB):
            blk = qp[i0:i0 + SLAB, :, tb * TIN:(tb + 1) * TIN, :]
            blk = blk.reshape(SLAB, W, TIN, NH, HD)
            # -> [j, d, i, t, h]
            q_sb[tb::TB] = np.ascontiguousarray(
                blk.transpose(1, 4, 0, 2, 3)).reshape(W, QF)
        # k_sb[jp*3+tb, (d, ip, tq, h)] = kp[i0+ip, jp, tb*14+tq, h*6+d]
        k_sb = np.zeros((KP, KF), np.float16)
        kc = kp[i0:i0 + SLAB + 2]
        for tb in range(TB):
            blk = kc[:, :, tb * TIN:tb * TIN + TQ, :]
            blk = blk.reshape(SLAB + 2, W + 2, TQ, NH, HD)
            k_sb[tb::TB] = np.ascontiguousarray(
                blk.transpose(1, 4, 0, 2, 3)).reshape(W + 2, KF)
        in_maps.append({"qs": q_sb, "ks": k_sb, "ws": w_rep})
    return in_maps


def _assemble(results):
    full = np.zeros((H, W, TB * TIN, NH, 3), np.float32)
    for c in range(N_CORES):
        i0 = c * SLAB
        o = results[c]["out"].reshape(W, TB, 3, SLAB, TIN, NH)
        # -> [i, j, tb, t, h, o]
        o = o.transpose(3, 0, 1, 4, 5, 2)
        full[i0:i0 + SLAB] = o.reshape(SLAB, W, TB * TIN, NH, 3)
    full = full[:, :, :T].reshape(H, W, T, NH * 3)
    return np.ascontiguousarray(full.transpose(3, 0, 1, 2))[None]


def _run(q, k, rpb, **spmd_kwargs):
    if "prog" not in _prog_cache:
        _prog_cache["prog"] = _build_program()
    nc = _prog_cache["prog"]
    in_maps = _host_prep(q, k, rpb)
    res = run_bass_kernel_spmd(nc, in_maps, list(range(N_CORES)),
                               **spmd_kwargs)
    return _assemble(res.results), res


def kernel(q, k, rpb):
    out, _ = _run(q, k, rpb)
    return out
